# revision 38
# baseline (speedup 1.0000x reference)
"""Dense MLP forward (y = quantize(relu(x @ w + b))) on 8 TRN2 NeuronCores.

Strategy: pure data-parallel over the batch dim (1024 rows per core), w/b
replicated, no collectives. Host-side each core receives its x shard
*transposed* so the contraction dim lands on SBUF partitions with contiguous
DMA — zero on-chip transposes. Each core computes yT tiles:

  - matmuls in bf16 (x and w both rounded host-side; adds ~3e-3 rel err vs
    the 2e-2 gate). bf16 halves x's HBM traffic vs f32 and LDWEIGHTS gets
    FWL (4-xbus fast weight load), so the LDW fully hides under the 213ns
    N=512 moving stream (measured 216ns/MM steady-state, vs 247ns for
    fp32r); w chunks [128k,128n] stationary, xT chunks [128k,512m] moving,
    accumulating over k into all 8 PSUM banks; k-major wave order in band 0
    so the PE starts as soon as the first chunks land; band 1 skewed (group
    nt runs chunk c at wave nt+c) so group stops stagger and evictions
    overlap matmuls.
  - startup: the framework's entry all-engine barrier is deleted from the
    IR; junk matmuls on uninitialized SBUF (N=256 bf16, the PE's first
    instructions) bridge the first-DMA window and hold the HAM activity
    window busy so the PE clock releases (1.2->2.4GHz) just before the
    real stream begins. Input DMAs spread across THREE queues (ACT + SP
    HWDGE rings, gpsimd SWDGE) interleaved in need-order: one queue moves
    only ~110-135 GB/s while several share the ~330 GB/s core wire, and
    the queues' start times differ (ACT ~7.7us, SWDGE/SP ~8.5-9.6us), so
    wave-0 gates ride the SWDGE/ACT queues and later waves alternate.
    Finer piece splits measure WORSE (per-queue throughput collapses).
  - epilogue per [128n, 512m] tile: relu(psum + b) in one op (bias is
    per-partition in the transposed layout), groups alternating ACT/DVE
    (cross-engine readers of ONE acc tile serialize, so each group gets a
    single full-width epilogue); group 7, the last stop, goes to ACT whose
    relu+bias op is fastest, group 6 to DVE, stores on disjoint rings.
  - exit: stock codegen runs TWO exit all-engine barriers and then a
    walrus NEFF epilogue where every engine serially clears ~50 semaphores
    (the slowest, PE, needs 6.4us) — all gated behind the final store DMA.
    The second barrier is deleted; ACT/PE/DVE are dropped from the first
    (the SP-side DMA-completion waits already fence Pool's RANGE_CLEAR,
    and the sems those engines clear are dead), so their clear streams
    start the moment their own work ends and overlap the store tail.

Host transposes each core's yT back and concatenates.
"""

import numpy as np
import ml_dtypes

import concourse.bacc as bacc
import concourse.tile as tile
from concourse import mybir
from concourse.bass_utils import run_bass_kernel_spmd

P = 128
B, D_IN, D_OUT = 8192, 1024, 1024
N_CORES = 8
M = B // N_CORES          # batch rows per core
KC = D_IN // P            # 8 k-chunks
NT = D_OUT // P           # 8 n-groups (PSUM partition tiles)
MB = 512                  # matmul moving free dim / PSUM bank width (fp32)
NUM_MB = M // MB          # 2 m-bands per core

N_WARMUP_MM = 19          # N=256 junk MMs filling the first-DMA window
DEBARRIER = True          # drop the entry-block all-engine barrier

F32 = mybir.dt.float32
BF16 = mybir.dt.bfloat16

_CACHE = {}


def build_bass(debarrier=DEBARRIER):
    nc = bacc.Bacc("TRN2", target_bir_lowering=False, debug=False)

    xT_d = nc.dram_tensor("xT", [D_IN, M], BF16, kind="ExternalInput")
    w_d = nc.dram_tensor("w", [D_IN, D_OUT], BF16, kind="ExternalInput")
    b_d = nc.dram_tensor("b", [D_OUT], F32, kind="ExternalInput")
    yT_d = nc.dram_tensor("yT", [D_OUT, M], BF16, kind="ExternalOutput")

    with tile.TileContext(nc) as tc:
        with (
            tc.tile_pool(name="const", bufs=1) as cst,
            tc.tile_pool(name="wx", bufs=1) as wx,
            tc.tile_pool(name="outp", bufs=16) as outp,
            tc.tile_pool(name="ps", bufs=1, space="PSUM") as ps,
        ):
            w_tiles = [wx.tile([P, D_OUT], BF16, tag=f"wc{c}", name=f"wc{c}") for c in range(KC)]
            x_tiles = [wx.tile([P, M], BF16, tag=f"xc{c}", name=f"xc{c}") for c in range(KC)]
            zt = cst.tile([P, 256], BF16, tag="warm_src")
            b_sb = cst.tile([P, NT], F32, tag="bias_raw")

            # ---- early ops ----
            # zt is junk-matmul fodder; it must be written once for the tile
            # allocator, so memset it on DVE (idle until the epilogues).
            nc.vector.memset(zt, 0.0)
            # A single DMA queue moves only ~135 GB/s while several are
            # active, so inputs spread across THREE queues (ACT + SP HWDGE,
            # gpsimd SWDGE) interleaved in need-order: each queue's local
            # order matches the k-major wave schedule so every wave's w
            # chunk and x band-0 piece land with slack. gpsimd leaves the
            # runtime preamble first, SP last (~1.3us later).
            def wpc(c, sl=slice(None)):
                return (w_tiles[c][:, sl], w_d.ap()[c * P : (c + 1) * P, sl])

            def xpc(c, sl):
                return (x_tiles[c][:, sl], xT_d.ap()[c * P : (c + 1) * P, sl])

            lo, hi = slice(None, MB), slice(MB, None)
            # Pieces interleave across the three queues in need-order; finer
            # splits measure WORSE (per-queue throughput collapses under
            # 3-way contention), so keep chunk-granularity pieces.
            gp_pieces = [xpc(0, lo), wpc(1), xpc(2, lo), wpc(4), xpc(5, lo), wpc(7), xpc(7, lo)]
            act_pieces = [wpc(0, lo), wpc(0, hi), wpc(2), xpc(3, lo), wpc(6)]
            sp_pieces = [xpc(1, lo), wpc(3), xpc(4, lo), wpc(5), xpc(6, lo)] + [
                xpc(c, hi) for c in range(KC)
            ]
            for out, in_ in gp_pieces:
                nc.gpsimd.dma_start(out=out, in_=in_)
            for out, in_ in act_pieces:
                nc.scalar.dma_start(out=out, in_=in_)
            nc.scalar.dma_start(out=b_sb, in_=b_d.ap().rearrange("(c p) -> p c", p=P))

            # PE warm-up on junk data (uninitialized SBUF — the junk PSUM
            # output is overwritten with start=True, so values don't matter)
            # while the first input DMAs stream in.
            warm_ps = ps.tile([P, MB], F32, tag="acc0")
            for _ in range(N_WARMUP_MM):
                nc.tensor.matmul(warm_ps[:, :256], zt[:, :P], zt, start=True, stop=True)

            for out, in_ in sp_pieces:
                nc.sync.dma_start(out=out, in_=in_)

            def make_accs():
                return [
                    ps.tile([P, MB], F32, tag=f"acc{nt}", name=f"acc{nt}")
                    for nt in range(NT)
                ]

            def emit_mm(accs, mb, nt, c):
                nc.tensor.matmul(
                    accs[nt],
                    w_tiles[c][:, nt * P : (nt + 1) * P],
                    x_tiles[c][:, mb * MB : (mb + 1) * MB],
                    start=(c == 0),
                    stop=(c == KC - 1),
                )

            def emit_epi(acc_sl, nt, on_act, o_sl):
                # relu(psum + b) -> bf16; bias varies along partitions here
                if on_act:
                    nc.scalar.activation(
                        o_sl, acc_sl, mybir.ActivationFunctionType.Relu,
                        bias=b_sb[:, nt : nt + 1], scale=1.0,
                    )
                else:
                    nc.vector.tensor_scalar(
                        o_sl, acc_sl, b_sb[:, nt : nt + 1], 0.0,
                        mybir.AluOpType.add, mybir.AluOpType.max,
                    )

            def emit_tail(accs, mb):
                # One full-width epilogue + one full-width store per group
                # (cross-engine readers of one acc tile serialize, so halves
                # don't parallelize): g7 on ACT (fastest epi op) + scalar
                # ring, g6 on DVE + sync ring, so the two final chains are
                # engine- and ring-disjoint.
                base = mb * MB
                for nt in range(NT):
                    on_act = (nt % 2 == 0) if nt < 6 else (nt == 7)
                    o = outp.tile([P, MB], BF16, tag="otile")
                    emit_epi(accs[nt], nt, on_act, o)
                    ring = nc.scalar if on_act else nc.sync
                    ring.dma_start(
                        out=yT_d.ap()[nt * P : (nt + 1) * P, base : base + MB],
                        in_=o,
                    )

            # ---- band 0: k-major waves, one chunk's MMs per arriving chunk ----
            accs = make_accs()
            for c in range(KC):
                for nt in range(NT):
                    emit_mm(accs, 0, nt, c)
            emit_tail(accs, 0)

            # ---- band 1: skewed waves (group nt runs chunk c at wave
            # t=nt+c) so stops stagger and evictions overlap matmuls ----
            accs = make_accs()
            for t in range(KC + NT - 1):
                for nt in range(NT):
                    c = t - nt
                    if 0 <= c < KC:
                        emit_mm(accs, 1, nt, c)
            emit_tail(accs, 1)

    if debarrier:
        # Drop the framework's entry all-engine barrier: every real
        # dependency already has a tile-emitted semaphore, and the barrier
        # serializes all engines behind the slowest pre-barrier stream.
        entry = nc.main_func.blocks[0]
        drop = [
            inst for inst in entry.instructions
            if type(inst).__name__ in ("InstDrain", "InstEventSemaphore")
        ]
        assert len(drop) == 11, [str(i)[:60] for i in drop]
        n_bar = sum("barrier_" in str(i) for i in drop)
        assert n_bar == 10, n_bar  # 5x(Drain+EvtSem w/ barrier sem) + bare PL Drain
        for inst in drop:
            entry.instructions.remove(inst)

        # Exit-path surgery. The walrus NEFF epilogue makes each engine
        # individually clear ~50 semaphores (45-130ns apiece, 2.5-6.5us per
        # engine) after our last block; stock codegen fences ALL of that
        # behind an all-engine barrier that waits for the final store DMAs,
        # then repeats the barrier a second time. Three facts make most of
        # that serialization removable:
        #   1. the SP-side completion waits (DMAHW*/DMASW* counters plus
        #      the PE/DVE/ACT op-count sems) already transitively fence
        #      every engine's last user op before Pool's tile RANGE_CLEAR;
        #   2. the sem ranges ACT/PE/DVE clear in the walrus epilogue
        #      (7-150) are never touched during this kernel's execution;
        #   3. the walrus epilogue ends in its own all-clear rendezvous.
        # So: drop barrier round 2 entirely, drop ACT/PE/DVE from round 1
        # (their epilogue clears then overlap the store tail), and shrink
        # Pool's gather/release counts to the one remaining participant
        # (SP).
        end = nc.main_func.blocks[-1]
        isa_idx = max(
            i for i, inst in enumerate(end.instructions)
            if type(inst).__name__ == "InstISA"
        )
        tail = end.instructions[isa_idx + 1 :]
        assert len(tail) == 11, [str(i)[:60] for i in tail]
        assert all(
            type(i).__name__ in ("InstDrain", "InstEventSemaphore") for i in tail
        ), [str(i)[:60] for i in tail]
        n_bar2 = sum("barrier_" in str(i) for i in tail)
        assert n_bar2 == 10, n_bar2
        for inst in tail:
            end.instructions.remove(inst)

        from concourse import mybir as _mybir

        skip = (
            _mybir.EngineType.Activation,
            _mybir.EngineType.PE,
            _mybir.EngineType.DVE,
        )
        drop1 = [
            inst for inst in end.instructions
            if inst.engine in skip and "barrier_" in str(inst)
            and type(inst).__name__ in ("InstDrain", "InstEventSemaphore")
        ]
        assert len(drop1) == 6, [str(i)[:60] for i in drop1]
        for inst in drop1:
            end.instructions.remove(inst)
        n_patched = 0
        for inst in end.instructions:
            if inst.engine != _mybir.EngineType.Pool or inst.sync_info is None:
                continue
            si = inst.sync_info
            for w in si.on_wait:
                if "gather" in w.ant_name and w.wait_value == 4:
                    w.wait_value = 1
                    n_patched += 1
            for u in si.on_update:
                if "gather" in u.ant_name and u.update_value == 4:
                    u.update_value = 1
                    n_patched += 1
                if "release" in u.ant_name and u.update_value == 4:
                    u.update_value = 1
                    n_patched += 1
        assert n_patched == 3, n_patched

    nc.compile()
    return nc


def get_nc():
    if "nc" not in _CACHE:
        _CACHE["nc"] = build_bass()
    return _CACHE["nc"]


def make_in_maps(x, w, b):
    x = np.asarray(x, dtype=np.float32)
    w = np.asarray(w, dtype=np.float32)
    b = np.ascontiguousarray(b, dtype=np.float32)
    w_bf = np.ascontiguousarray(w.astype(ml_dtypes.bfloat16))
    xs = x.reshape(N_CORES, M, D_IN)
    return [
        {
            "xT": np.ascontiguousarray(xs[i].T.astype(ml_dtypes.bfloat16)),
            "w": w_bf,
            "b": b,
        }
        for i in range(N_CORES)
    ]


def gather_out(results):
    return np.concatenate(
        [results[i]["yT"].astype(np.float32).T for i in range(N_CORES)], axis=0
    )


def kernel(x, w, b):
    nc = get_nc()
    res = run_bass_kernel_spmd(nc, make_in_maps(x, w, b), core_ids=list(range(N_CORES)))
    return gather_out(res.results)
